# revision 26
# baseline (speedup 1.0000x reference)
"""Trainium2 Bass kernel for dual-softmax attention (nn_Attention_67284957659466).

Computation (per reference):
    qkv = x @ w_qkv + b_qkv            x: (B=16, N=596, C=768), w_qkv: (768, 2304)
    q, k, v per head (H=12, Dh=64)
    S = q @ k^T * Dh^-0.5              (B, H, N, N)
    A = concat(softmax(S[..., :20]), softmax(S[..., 20:]))   # dual-group softmax
    out = (A @ v) reshaped -> (B, N, C) @ w_proj + b_proj
    returns (out, A)

Sharding: data-parallel over batch, 2 batches per core across 8 cores.

Per-core layout strategy (all matmuls in fp32r, 1 cyc/row at free>=256):
  - x^T (c on partitions) via PE transpose; qkv matmul gives q^T,k^T
    (c_out on partitions, n free) and v in natural layout (n on partitions).
  - Natural-orientation scores S (i part, j free) -> exp with accum_out
    row sums -> per-partition normalize -> weights output (contiguous DMA).
  - Transposed scores S^T (j part, i free) -> exp -> A^T unnormalized (ET);
    A@V as out^T = v_aug^T.T @ ET with a ones column in v_aug giving the
    group denominators as an extra psum row; normalize via row-reciprocal +
    gpsimd partition_broadcast; out^T feeds proj directly as lhsT.
  - proj gives y in natural (i part, c free) layout for contiguous DMA.

b_qkv's v-part and b_proj are folded in on the host:
  y += (2 * b_v) @ w_proj + b_proj   (each softmax group's A rows sum to 1).
"""

import sys

for _p in ("/opt/pypackages", "/opt/trn_rl_repo"):
    if _p not in sys.path:
        sys.path.insert(0, _p)

import numpy as np

import concourse.bacc as bacc
import concourse.bass as bass
import concourse.mybir as mybir
import concourse.tile as tile
from concourse.bass_utils import run_bass_kernel_spmd
from concourse.masks import make_identity

B, N, C = 16, 596, 768
H, Dh = 12, 64
NCLS = 20
NCORES = 8
BL = B // NCORES  # batches per core
SCALE = float(Dh) ** -0.5

f32 = mybir.dt.float32
f32r = mybir.dt.float32r
EXP = mybir.ActivationFunctionType.Exp
MULT = mybir.AluOpType.mult

NT = [128, 128, 128, 128, 84]          # 596 = 4*128 + 84 (i/j/n tiling)
NSTART = [0, 128, 256, 384, 512]
CHUNKS = [(0, 298), (298, 298)]        # free-dim chunks of 596 (>=256 for fp32r)
CT = C // 128                          # 6 c-tiles
MT_QK = 2 * C // 128                   # 12 c_out m-tiles covering q,k (1536)
PCH = [(0, 384), (384, 384)]           # 768-wide free chunks


def _emit(nc, tc, x_d, wqkv_d, bqkv_d, wproj_d, sel_d, out_d, weights_d):
    import contextlib

    ctx = contextlib.ExitStack()
    with ctx:
        consts = ctx.enter_context(tc.tile_pool(name="consts", bufs=1))
        ps = ctx.enter_context(tc.tile_pool(name="ps", bufs=8, space="PSUM"))

        ident = consts.tile([128, 128], f32)
        make_identity(nc, ident[:])
        # selector matrices (host-provided): sel_r[:, g, :] broadcasts
        # reciprocal row g of R4=[rc0, rp0, rc1, rp1] to all 128 partitions
        sel_f = consts.tile([4, 4, 128], f32)
        nc.sync.dma_start(sel_f[:], sel_d[:])
        sel_r = consts.tile([4, 4, 128], f32r)
        nc.vector.tensor_copy(sel_r[:], sel_f[:])

        # ---- load + round weights to fp32r ----
        w_r = consts.tile([128, CT, 3 * C], f32r)      # w_qkv, c_in on partitions
        wp_r = consts.tile([128, CT, C], f32r)         # w_proj
        with tc.tile_pool(name="wstage", bufs=2) as wstage:
            for ctile in range(CT):
                stg = wstage.tile([128, 3 * C], f32, tag="wstg")
                nc.sync.dma_start(stg[:], wqkv_d[ctile * 128:(ctile + 1) * 128, :])
                nc.scalar.copy(w_r[:, ctile, :], stg[:])
            for ctile in range(CT):
                stg = wstage.tile([128, C], f32, tag="wstg")
                nc.sync.dma_start(stg[:], wproj_d[ctile * 128:(ctile + 1) * 128, :])
                nc.scalar.copy(wp_r[:, ctile, :], stg[:])

        # q,k bias: per-partition scalars (c_out on partitions), column m = m-tile
        bias_qk = consts.tile([128, MT_QK], f32)
        with nc.allow_non_contiguous_dma(reason="tiny strided bias load (1536 elems)"):
            nc.sync.dma_start(
                bias_qk[:],
                bqkv_d[0:2 * C].rearrange("(o p) -> p o", p=128),
            )

        # ---- persistent per-batch buffers ----
        qkT_r = consts.tile([128, MT_QK, N], f32r)     # q^T,k^T (c_out part, n free)
        v_r = consts.tile([128, 5, H * Dh + 1], f32r)  # v natural (+1 spare col)
        v0_r = consts.tile([128, H * Dh + 1], f32r)    # j-tile 0 copy, rows 0:20 zeroed
        outT = consts.tile([128, CT, N], f32)          # attention out^T (c part, i free)

        # xT lives only through the QKV phase; the big per-head pools only
        # through the attention phase -- scope them per batch so both fit
        for b in range(BL):
          with tc.tile_pool(name="xstage", bufs=2) as xstage, \
               tc.tile_pool(name="xtp", bufs=1) as xtp:
            xT_r = xtp.tile([128, CT, N], f32r, tag="xT")
            # ---- x^T via PE transpose ----
            for it in range(5):
                ns = NT[it]
                xs = xstage.tile([128, C], f32, tag="xs")
                nc.sync.dma_start(xs[0:ns, :], x_d[b, NSTART[it]:NSTART[it] + ns, :])
                for ctile in range(CT):
                    pt = ps.tile([128, 384], f32, tag="ps")
                    nc.tensor.transpose(
                        pt[:, 0:ns], xs[0:ns, ctile * 128:(ctile + 1) * 128],
                        ident[0:ns, 0:ns]
                    )
                    nc.scalar.copy(
                        xT_r[:, ctile, NSTART[it]:NSTART[it] + ns], pt[:, 0:ns]
                    )

            # ---- qkv: q^T, k^T (c_out on partitions) ----
            for m in [mm for p in range(MT_QK // 2) for mm in (p, MT_QK // 2 + p)]:
                for (c0, cl) in CHUNKS:
                    pt = ps.tile([128, 384], f32, tag="ps")
                    for kk in range(CT):
                        nc.tensor.matmul(
                            pt[:, 0:cl],
                            w_r[:, kk, m * 128:(m + 1) * 128],
                            xT_r[:, kk, c0:c0 + cl],
                            start=(kk == 0),
                            stop=(kk == CT - 1),
                        )
                    nc.scalar.add(
                        qkT_r[:, m, c0:c0 + cl], pt[:, 0:cl], bias_qk[:, m:m + 1]
                    )

            # ---- qkv: v natural (n on partitions, c_out free) ----
            for it in range(5):
                ns = NT[it]
                for (p0, pl) in PCH:
                    pt = ps.tile([128, 384], f32, tag="ps")
                    for kk in range(CT):
                        nc.tensor.matmul(
                            pt[0:ns, 0:pl],
                            xT_r[:, kk, NSTART[it]:NSTART[it] + ns],
                            w_r[:, kk, 2 * C + p0:2 * C + p0 + pl],
                            start=(kk == 0),
                            stop=(kk == CT - 1),
                        )
                    nc.scalar.copy(
                        v_r[0:ns, it, p0:p0 + pl], pt[0:ns, 0:pl]
                    )
            nc.vector.tensor_copy(v0_r[:], v_r[:, 0, :])
            nc.vector.tensor_scalar_mul(v0_r[0:NCLS, :], v0_r[0:NCLS, :], 0.0)

            # ---- natural orientation, head pairs (row-packed matmuls):
            # weights output + softmax denominators ----
            rcp_all = {}
            for hp in range(H // 2):
                hq = hp                   # q c-tile (heads 2hp, 2hp+1)
                hk = MT_QK // 2 + hp      # k c-tile
                for it in range(5):
                    ns = NT[it]
                    e_pair = [enat.tile([128, N], f32, tag="enat", name=f"e{_hx}") for _hx in range(2)]
                    # layout: [dc0, dpa0, dc1, dpa1, dpb0, dpb1] so the
                    # reciprocal reads a contiguous (128, 4) block
                    dsum = small.tile([128, 8], f32, tag="dsum")
                    pts = {}
                    # interleave the two heads' matmuls: row groups 0-63 and
                    # 64-127 run concurrently in the PE array; tile_critical
                    # keeps each pair adjacent in the PE stream
                    for ci, (c0, cl) in enumerate(CHUNKS):
                        for hx in range(2):
                            pts[(hx, ci)] = ps.tile([128, 384], f32, tag="ps", name=f"pt{hx}{ci}")
                        for hx in range(2):
                            off = hx * 64
                            nc.tensor.matmul(
                                pts[(hx, ci)][0:ns, 0:cl],
                                qkT_r[off:off + 64, hq, NSTART[it]:NSTART[it] + ns],
                                qkT_r[off:off + 64, hk, c0:c0 + cl],
                                start=True,
                                stop=True,
                            )
                    for hx in range(2):
                        e = e_pair[hx]
                        nc.scalar.activation(
                            e[0:ns, 0:NCLS], pts[(hx, 0)][0:ns, 0:NCLS], EXP,
                            scale=SCALE, accum_out=dsum[0:ns, 2 * hx:2 * hx + 1],
                        )
                        nc.scalar.activation(
                            e[0:ns, NCLS:298], pts[(hx, 0)][0:ns, NCLS:298], EXP,
                            scale=SCALE, accum_out=dsum[0:ns, 2 * hx + 1:2 * hx + 2],
                        )
                        nc.scalar.activation(
                            e[0:ns, 298:N], pts[(hx, 1)][0:ns, 0:298], EXP,
                            scale=SCALE, accum_out=dsum[0:ns, 4 + hx:5 + hx],
                        )
                    # batched denominators: two adds + one contiguous reciprocal
                    nc.vector.tensor_add(
                        dsum[0:ns, 1:2], dsum[0:ns, 1:2], dsum[0:ns, 4:5]
                    )
                    nc.vector.tensor_add(
                        dsum[0:ns, 3:4], dsum[0:ns, 3:4], dsum[0:ns, 5:6]
                    )
                    rcp = small.tile([128, 4], f32, tag="rcp")
                    nc.vector.reciprocal(rcp[0:ns, :], dsum[0:ns, 0:4])
                    rcp_all[(hp, it)] = rcp
                    for hx in range(2):
                        e = e_pair[hx]
                        nc.vector.tensor_scalar_mul(
                            e[0:ns, 0:NCLS], e[0:ns, 0:NCLS], rcp[0:ns, 2 * hx:2 * hx + 1]
                        )
                        nc.vector.tensor_scalar_mul(
                            e[0:ns, NCLS:N], e[0:ns, NCLS:N], rcp[0:ns, 2 * hx + 1:2 * hx + 2]
                        )
                        nc.sync.dma_start(
                            weights_d[b, 2 * hp + hx, NSTART[it]:NSTART[it] + ns, :],
                            e[0:ns, :],
                        )

            # ---- transposed orientation + A@V, per head pair ----
            for hp in range(H // 2):
                hq = hp
                hk = MT_QK // 2 + hp

                # reciprocal rows R4 = [rc0, rp0, rc1, rp1] (i on free dim)
                # via PE transpose of the natural-path reciprocals
                r4 = rows.tile([4, N], f32r, tag="r4")
                for it in range(5):
                    ns = NT[it]
                    ptr = ps.tile([128, 384], f32, tag="ps")
                    nc.tensor.transpose(
                        ptr[0:4, 0:ns], rcp_all.pop((hp, it))[0:ns, 0:4],
                        ident[0:ns, 0:ns]
                    )
                    nc.scalar.copy(r4[:, NSTART[it]:NSTART[it] + ns], ptr[0:4, 0:ns])

                # S^T pairs (row-packed) -> exp -> ET (unnormalized, f32r)
                et_tiles = {}
                for jt in range(5):
                    njs = NT[jt]
                    for hx in range(2):
                        et_tiles[(hx, jt)] = et.tile([128, N], f32r, tag="et", name=f"et{hx}")
                    for (c0, cl) in CHUNKS:
                        pps = [ps.tile([128, 384], f32, tag="ps", name=f"pst{hx}") for hx in range(2)]
                        for hx in range(2):
                            off = hx * 64
                            nc.tensor.matmul(
                                pps[hx][0:njs, 0:cl],
                                qkT_r[off:off + 64, hk, NSTART[jt]:NSTART[jt] + njs],
                                qkT_r[off:off + 64, hq, c0:c0 + cl],
                                start=True,
                                stop=True,
                            )
                        for hx in range(2):
                            nc.scalar.activation(
                                et_tiles[(hx, jt)][0:njs, c0:c0 + cl],
                                pps[hx][0:njs, 0:cl], EXP, scale=SCALE,
                            )

                # A@V per head (m=65 via overhang column keeps the PE tile
                # at legal 128-col size); normalization rows broadcast via
                # selector matmuls from R4
                for (c0, cl) in CHUNKS:
                    bcs = []
                    for hx in range(2):
                        for g in range(2):
                            gi = 2 * hx + g if g == 0 else 2 * hx + 1
                            gi = 2 * hx + g
                            bps = ps.tile([128, 384], f32, tag="ps", name=f"bps{hx}{g}")
                            nc.tensor.matmul(
                                bps[:, 0:cl], sel_r[:, gi, :], r4[:, c0:c0 + cl],
                                start=True, stop=True,
                            )
                            bc = bcast.tile([128, 298], f32, tag="bcast", name=f"bc{hx}{g}")
                            nc.vector.tensor_copy(bc[:, 0:cl], bps[:, 0:cl])
                            bcs.append(bc)
                    for hx in range(2):
                        h = 2 * hp + hx
                        p1 = ps.tile([128, 384], f32, tag="ps", name=f"p1{hx}")
                        p2 = ps.tile([128, 384], f32, tag="ps", name=f"p2{hx}")
                        nc.tensor.matmul(
                            p1[0:Dh + 1, 0:cl],
                            v_r[0:NCLS, 0, h * Dh:h * Dh + Dh + 1],
                            et_tiles[(hx, 0)][0:NCLS, c0:c0 + cl],
                            start=True,
                            stop=True,
                        )
                        for jt in range(5):
                            njs = NT[jt]
                            lhs = (v0_r[:, h * Dh:h * Dh + Dh + 1] if jt == 0
                                   else v_r[0:njs, jt, h * Dh:h * Dh + Dh + 1])
                            nc.tensor.matmul(
                                p2[0:Dh + 1, 0:cl],
                                lhs,
                                et_tiles[(hx, jt)][0:njs, c0:c0 + cl],
                                start=(jt == 0),
                                stop=(jt == 4),
                            )
                        bcc, bcp = bcs[2 * hx], bcs[2 * hx + 1]
                        tm = tmpp.tile([128, 298], f32, tag="tmp")
                        nc.vector.tensor_tensor(
                            tm[0:Dh, 0:cl], p2[0:Dh, 0:cl], bcp[0:Dh, 0:cl], MULT
                        )
                        if hx == 0:
                            oslice = outT[0:Dh, hq, c0:c0 + cl]
                            nc.vector.tensor_tensor(
                                oslice, p1[0:Dh, 0:cl], bcc[0:Dh, 0:cl], MULT
                            )
                            nc.vector.tensor_add(oslice, oslice, tm[0:Dh, 0:cl])
                        else:
                            tm2 = tmpp.tile([128, 298], f32, tag="tmp2")
                            nc.vector.tensor_tensor(
                                tm2[0:Dh, 0:cl], p1[0:Dh, 0:cl], bcc[0:Dh, 0:cl], MULT
                            )
                            nc.vector.tensor_add(
                                tm2[0:Dh, 0:cl], tm2[0:Dh, 0:cl], tm[0:Dh, 0:cl]
                            )
                            nc.sync.dma_start(
                                outT[64:128, hq, c0:c0 + cl], tm2[0:Dh, 0:cl]
                            )

            # ---- proj: y = out^T.T @ w_proj (natural layout out) ----
            # round outT slices to fp32r on the fly for the matmul lhsT
            for it in range(5):
                ns = NT[it]
                y = ypool.tile([128, C], f32, tag="y")
                lhs_tiles = []
                for kk in range(CT):
                    lr = lhsr.tile([128, 128], f32r, tag="lhsr")
                    nc.vector.tensor_copy(
                        lr[:, 0:ns], outT[:, kk, NSTART[it]:NSTART[it] + ns]
                    )
                    lhs_tiles.append(lr)
                for (p0, pl) in PCH:
                    pt = ps.tile([128, 384], f32, tag="ps")
                    for kk in range(CT):
                        nc.tensor.matmul(
                            pt[0:ns, 0:pl],
                            lhs_tiles[kk][:, 0:ns],
                            wp_r[:, kk, p0:p0 + pl],
                            start=(kk == 0),
                            stop=(kk == CT - 1),
                        )
                    nc.scalar.copy(y[0:ns, p0:p0 + pl], pt[0:ns, 0:pl])
                nc.sync.dma_start(out_d[b, NSTART[it]:NSTART[it] + ns, :], y[0:ns, :])


def build_program():
    nc = bacc.Bacc()
    x_d = nc.declare_dram_parameter("x", [BL, N, C], f32, isOutput=False)
    wqkv_d = nc.declare_dram_parameter("w_qkv", [C, 3 * C], f32, isOutput=False)
    bqkv_d = nc.declare_dram_parameter("b_qkv", [3 * C], f32, isOutput=False)
    wproj_d = nc.declare_dram_parameter("w_proj", [C, C], f32, isOutput=False)
    sel_d = nc.declare_dram_parameter("sel", [4, 4, 128], f32, isOutput=False)
    out_d = nc.declare_dram_parameter("out", [BL, N, C], f32, isOutput=True)
    weights_d = nc.declare_dram_parameter("weights", [BL, H, N, N], f32, isOutput=True)

    with tile.TileContext(nc) as tc:
        _emit(nc, tc, x_d, wqkv_d, bqkv_d, wproj_d, sel_d, out_d, weights_d)
    nc.compile()
    return nc


_PROGRAM = None


def _get_program():
    global _PROGRAM
    if _PROGRAM is None:
        _PROGRAM = build_program()
    return _PROGRAM


def run(x, w_qkv, b_qkv, w_proj, b_proj, trace=False, **trace_kwargs):
    """Run on 8 cores; returns (out, weights, BassKernelResults)."""
    x = np.ascontiguousarray(np.asarray(x, dtype=np.float32))
    w_qkv = np.ascontiguousarray(np.asarray(w_qkv, dtype=np.float32))
    b_qkv = np.ascontiguousarray(np.asarray(b_qkv, dtype=np.float32))
    w_proj = np.ascontiguousarray(np.asarray(w_proj, dtype=np.float32))
    b_proj = np.ascontiguousarray(np.asarray(b_proj, dtype=np.float32))

    nc = _get_program()
    sel = np.zeros((4, 4, 128), dtype=np.float32)
    for g in range(4):
        sel[g, g, :] = 1.0
    core_ids = list(range(NCORES))
    in_maps = [
        {
            "x": x[i * BL:(i + 1) * BL],
            "w_qkv": w_qkv,
            "b_qkv": b_qkv,
            "w_proj": w_proj,
            "sel": sel,
        }
        for i in core_ids
    ]
    res = run_bass_kernel_spmd(nc, in_maps, core_ids, trace=trace, **trace_kwargs)
    out = np.concatenate([r["out"] for r in res.results], axis=0)
    weights = np.concatenate([r["weights"] for r in res.results], axis=0)
    # host-side fold of the v-bias and proj bias (see module docstring)
    bias = 2.0 * (b_qkv[2 * C:] @ w_proj) + b_proj
    if np.any(bias != 0.0):
        out = out + bias[None, None, :]
    return out, weights, res


def kernel(x, w_qkv, b_qkv, w_proj, b_proj, num_heads, num_cls):
    assert int(num_heads) == H and int(num_cls) == NCLS, (num_heads, num_cls)
    out, weights, _ = run(x, w_qkv, b_qkv, w_proj, b_proj)
    return out, weights


# revision 27
# speedup vs baseline: 1.0889x; 1.0889x over previous
"""Trainium2 Bass kernel for dual-softmax attention (nn_Attention_67284957659466).

Computation (per reference):
    qkv = x @ w_qkv + b_qkv            x: (B=16, N=596, C=768), w_qkv: (768, 2304)
    q, k, v per head (H=12, Dh=64)
    S = q @ k^T * Dh^-0.5              (B, H, N, N)
    A = concat(softmax(S[..., :20]), softmax(S[..., 20:]))   # dual-group softmax
    out = (A @ v) reshaped -> (B, N, C) @ w_proj + b_proj
    returns (out, A)

Sharding: data-parallel over batch, 2 batches per core across 8 cores.

Per-core layout strategy (all matmuls in fp32r, 1 cyc/row at free>=256):
  - x^T (c on partitions) via PE transpose; qkv matmul gives q^T,k^T
    (c_out on partitions, n free) and v in natural layout (n on partitions).
  - Natural-orientation scores S (i part, j free) -> exp with accum_out
    row sums -> per-partition normalize -> weights output (contiguous DMA).
  - Transposed scores S^T (j part, i free) -> exp -> A^T unnormalized (ET);
    A@V as out^T = v_aug^T.T @ ET with a ones column in v_aug giving the
    group denominators as an extra psum row; normalize via row-reciprocal +
    gpsimd partition_broadcast; out^T feeds proj directly as lhsT.
  - proj gives y in natural (i part, c free) layout for contiguous DMA.

b_qkv's v-part and b_proj are folded in on the host:
  y += (2 * b_v) @ w_proj + b_proj   (each softmax group's A rows sum to 1).
"""

import sys

for _p in ("/opt/pypackages", "/opt/trn_rl_repo"):
    if _p not in sys.path:
        sys.path.insert(0, _p)

import numpy as np

import concourse.bacc as bacc
import concourse.bass as bass
import concourse.mybir as mybir
import concourse.tile as tile
from concourse.bass_utils import run_bass_kernel_spmd
from concourse.masks import make_identity

B, N, C = 16, 596, 768
H, Dh = 12, 64
NCLS = 20
NCORES = 8
BL = B // NCORES  # batches per core
SCALE = float(Dh) ** -0.5

f32 = mybir.dt.float32
f32r = mybir.dt.float32r
EXP = mybir.ActivationFunctionType.Exp
MULT = mybir.AluOpType.mult
bf16 = mybir.dt.bfloat16

NT = [128, 128, 128, 128, 84]          # 596 = 4*128 + 84 (i/j/n tiling)
NSTART = [0, 128, 256, 384, 512]
CHUNKS = [(0, 298), (298, 298)]        # free-dim chunks of 596 (>=256 for fp32r)
CT = C // 128                          # 6 c-tiles
MT_QK = 2 * C // 128                   # 12 c_out m-tiles covering q,k (1536)
PCH = [(0, 384), (384, 384)]           # 768-wide free chunks


def _emit(nc, tc, x_d, wqkv_d, bqkv_d, wproj_d, sel_d, out_d, weights_d):
    import contextlib

    ctx = contextlib.ExitStack()
    with ctx:
        consts = ctx.enter_context(tc.tile_pool(name="consts", bufs=1))
        ps = ctx.enter_context(tc.tile_pool(name="ps", bufs=8, space="PSUM"))

        ident = consts.tile([128, 128], f32)
        make_identity(nc, ident[:])
        # selector matrices (host-provided): sel_r[:, g, :] broadcasts
        # reciprocal row g of R4=[rc0, rp0, rc1, rp1] to all 128 partitions
        sel_f = consts.tile([4, 4, 128], f32)
        nc.sync.dma_start(sel_f[:], sel_d[:])
        sel_r = consts.tile([4, 4, 128], f32r)
        nc.vector.tensor_copy(sel_r[:], sel_f[:])

        # ---- load + round weights to fp32r ----
        w_r = consts.tile([128, CT, 3 * C], f32r)      # w_qkv, c_in on partitions
        wp_r = consts.tile([128, CT, C], f32r)         # w_proj
        with tc.tile_pool(name="wstage", bufs=2) as wstage:
            for ctile in range(CT):
                stg = wstage.tile([128, 3 * C], f32, tag="wstg")
                nc.sync.dma_start(stg[:], wqkv_d[ctile * 128:(ctile + 1) * 128, :])
                nc.scalar.copy(w_r[:, ctile, :], stg[:])
            for ctile in range(CT):
                stg = wstage.tile([128, C], f32, tag="wstg")
                nc.sync.dma_start(stg[:], wproj_d[ctile * 128:(ctile + 1) * 128, :])
                nc.scalar.copy(wp_r[:, ctile, :], stg[:])

        # q,k bias: per-partition scalars (c_out on partitions), column m = m-tile
        bias_qk = consts.tile([128, MT_QK], f32)
        with nc.allow_non_contiguous_dma(reason="tiny strided bias load (1536 elems)"):
            nc.sync.dma_start(
                bias_qk[:],
                bqkv_d[0:2 * C].rearrange("(o p) -> p o", p=128),
            )

        # ---- persistent per-batch buffers ----
        qkT_r = consts.tile([128, MT_QK, N], f32r)     # q^T,k^T (c_out part, n free)
        v_r = consts.tile([128, 5, H * Dh + 1], bf16)  # v natural (+1 spare col)
        v0_r = consts.tile([128, H * Dh + 1], bf16)    # j-tile 0 copy, rows 0:20 zeroed
        outT = consts.tile([128, CT, N], f32)          # attention out^T (c part, i free)

        # xT lives only through the QKV phase; the big per-head pools only
        # through the attention phase -- scope them per batch so both fit
        for b in range(BL):
          with tc.tile_pool(name="xstage", bufs=2) as xstage, \
               tc.tile_pool(name="xtp", bufs=1) as xtp:
            xT_r = xtp.tile([128, CT, N], f32r, tag="xT")
            # ---- x^T via PE transpose ----
            for it in range(5):
                ns = NT[it]
                xs = xstage.tile([128, C], f32, tag="xs")
                nc.sync.dma_start(xs[0:ns, :], x_d[b, NSTART[it]:NSTART[it] + ns, :])
                for ctile in range(CT):
                    pt = ps.tile([128, 384], f32, tag="ps")
                    nc.tensor.transpose(
                        pt[:, 0:ns], xs[0:ns, ctile * 128:(ctile + 1) * 128],
                        ident[0:ns, 0:ns]
                    )
                    nc.scalar.copy(
                        xT_r[:, ctile, NSTART[it]:NSTART[it] + ns], pt[:, 0:ns]
                    )

            # ---- qkv: q^T, k^T (c_out on partitions) ----
            for m in [mm for p in range(MT_QK // 2) for mm in (p, MT_QK // 2 + p)]:
                for (c0, cl) in CHUNKS:
                    pt = ps.tile([128, 384], f32, tag="ps")
                    for kk in range(CT):
                        nc.tensor.matmul(
                            pt[:, 0:cl],
                            w_r[:, kk, m * 128:(m + 1) * 128],
                            xT_r[:, kk, c0:c0 + cl],
                            start=(kk == 0),
                            stop=(kk == CT - 1),
                        )
                    nc.scalar.add(
                        qkT_r[:, m, c0:c0 + cl], pt[:, 0:cl], bias_qk[:, m:m + 1]
                    )

            # ---- qkv: v natural (n on partitions, c_out free) ----
            for it in range(5):
                ns = NT[it]
                for (p0, pl) in PCH:
                    pt = ps.tile([128, 384], f32, tag="ps")
                    for kk in range(CT):
                        nc.tensor.matmul(
                            pt[0:ns, 0:pl],
                            xT_r[:, kk, NSTART[it]:NSTART[it] + ns],
                            w_r[:, kk, 2 * C + p0:2 * C + p0 + pl],
                            start=(kk == 0),
                            stop=(kk == CT - 1),
                        )
                    nc.scalar.copy(
                        v_r[0:ns, it, p0:p0 + pl], pt[0:ns, 0:pl]
                    )
            nc.vector.tensor_copy(v0_r[:], v_r[:, 0, :])
            nc.vector.tensor_scalar_mul(v0_r[0:NCLS, :], v0_r[0:NCLS, :], 0.0)

            # ---- natural orientation, head pairs (row-packed matmuls):
            # weights output + softmax denominators ----
            rcp_all = {}
            for hp in range(H // 2):
                hq = hp                   # q c-tile (heads 2hp, 2hp+1)
                hk = MT_QK // 2 + hp      # k c-tile
                for it in range(5):
                    ns = NT[it]
                    e_pair = [enat.tile([128, N], f32, tag="enat", name=f"e{_hx}") for _hx in range(2)]
                    # layout: [dc0, dpa0, dc1, dpa1, dpb0, dpb1] so the
                    # reciprocal reads a contiguous (128, 4) block
                    dsum = small.tile([128, 8], f32, tag="dsum")
                    pts = {}
                    # interleave the two heads' matmuls: row groups 0-63 and
                    # 64-127 run concurrently in the PE array; tile_critical
                    # keeps each pair adjacent in the PE stream
                    for ci, (c0, cl) in enumerate(CHUNKS):
                        for hx in range(2):
                            pts[(hx, ci)] = ps.tile([128, 384], f32, tag="ps", name=f"pt{hx}{ci}")
                        for hx in range(2):
                            off = hx * 64
                            nc.tensor.matmul(
                                pts[(hx, ci)][0:ns, 0:cl],
                                qkT_r[off:off + 64, hq, NSTART[it]:NSTART[it] + ns],
                                qkT_r[off:off + 64, hk, c0:c0 + cl],
                                start=True,
                                stop=True,
                            )
                    for hx in range(2):
                        e = e_pair[hx]
                        nc.scalar.activation(
                            e[0:ns, 0:NCLS], pts[(hx, 0)][0:ns, 0:NCLS], EXP,
                            scale=SCALE, accum_out=dsum[0:ns, 2 * hx:2 * hx + 1],
                        )
                        nc.scalar.activation(
                            e[0:ns, NCLS:298], pts[(hx, 0)][0:ns, NCLS:298], EXP,
                            scale=SCALE, accum_out=dsum[0:ns, 2 * hx + 1:2 * hx + 2],
                        )
                        nc.scalar.activation(
                            e[0:ns, 298:N], pts[(hx, 1)][0:ns, 0:298], EXP,
                            scale=SCALE, accum_out=dsum[0:ns, 4 + hx:5 + hx],
                        )
                    # batched denominators: two adds + one contiguous reciprocal
                    nc.vector.tensor_add(
                        dsum[0:ns, 1:2], dsum[0:ns, 1:2], dsum[0:ns, 4:5]
                    )
                    nc.vector.tensor_add(
                        dsum[0:ns, 3:4], dsum[0:ns, 3:4], dsum[0:ns, 5:6]
                    )
                    rcp = small.tile([128, 4], f32, tag="rcp")
                    nc.vector.reciprocal(rcp[0:ns, :], dsum[0:ns, 0:4])
                    rcp_all[(hp, it)] = rcp
                    for hx in range(2):
                        e = e_pair[hx]
                        nc.vector.tensor_scalar_mul(
                            e[0:ns, 0:NCLS], e[0:ns, 0:NCLS], rcp[0:ns, 2 * hx:2 * hx + 1]
                        )
                        nc.vector.tensor_scalar_mul(
                            e[0:ns, NCLS:N], e[0:ns, NCLS:N], rcp[0:ns, 2 * hx + 1:2 * hx + 2]
                        )
                        nc.sync.dma_start(
                            weights_d[b, 2 * hp + hx, NSTART[it]:NSTART[it] + ns, :],
                            e[0:ns, :],
                        )

            # ---- transposed orientation + A@V, per head pair ----
            for hp in range(H // 2):
                hq = hp
                hk = MT_QK // 2 + hp

                # reciprocal rows R4 = [rc0, rp0, rc1, rp1] (i on free dim)
                # via PE transpose of the natural-path reciprocals
                r4 = rows.tile([4, N], f32r, tag="r4")
                for it in range(5):
                    ns = NT[it]
                    ptr = ps.tile([128, 384], f32, tag="ps")
                    nc.tensor.transpose(
                        ptr[0:4, 0:ns], rcp_all.pop((hp, it))[0:ns, 0:4],
                        ident[0:ns, 0:ns]
                    )
                    nc.scalar.copy(r4[:, NSTART[it]:NSTART[it] + ns], ptr[0:4, 0:ns])

                # S^T pairs (row-packed) -> exp -> ET (unnormalized, f32r)
                et_tiles = {}
                for jt in range(5):
                    njs = NT[jt]
                    for hx in range(2):
                        et_tiles[(hx, jt)] = et.tile([128, N], bf16, tag="et", name=f"et{hx}")
                    for (c0, cl) in CHUNKS:
                        pps = [ps.tile([128, 384], f32, tag="ps", name=f"pst{hx}") for hx in range(2)]
                        for hx in range(2):
                            off = hx * 64
                            nc.tensor.matmul(
                                pps[hx][0:njs, 0:cl],
                                qkT_r[off:off + 64, hk, NSTART[jt]:NSTART[jt] + njs],
                                qkT_r[off:off + 64, hq, c0:c0 + cl],
                                start=True,
                                stop=True,
                            )
                        for hx in range(2):
                            nc.scalar.activation(
                                et_tiles[(hx, jt)][0:njs, c0:c0 + cl],
                                pps[hx][0:njs, 0:cl], EXP, scale=SCALE,
                            )

                # A@V per head (m=65 via overhang column keeps the PE tile
                # at legal 128-col size); normalization rows broadcast via
                # selector matmuls from R4
                for (c0, cl) in CHUNKS:
                    bcs = []
                    for hx in range(2):
                        for g in range(2):
                            gi = 2 * hx + g if g == 0 else 2 * hx + 1
                            gi = 2 * hx + g
                            bps = ps.tile([128, 384], f32, tag="ps", name=f"bps{hx}{g}")
                            nc.tensor.matmul(
                                bps[:, 0:cl], sel_r[:, gi, :], r4[:, c0:c0 + cl],
                                start=True, stop=True,
                            )
                            bc = bcast.tile([128, 298], f32, tag="bcast", name=f"bc{hx}{g}")
                            nc.vector.tensor_copy(bc[:, 0:cl], bps[:, 0:cl])
                            bcs.append(bc)
                    for hx in range(2):
                        h = 2 * hp + hx
                        p1 = ps.tile([128, 384], f32, tag="ps", name=f"p1{hx}")
                        p2 = ps.tile([128, 384], f32, tag="ps", name=f"p2{hx}")
                        nc.tensor.matmul(
                            p1[0:Dh + 1, 0:cl],
                            v_r[0:NCLS, 0, h * Dh:h * Dh + Dh + 1],
                            et_tiles[(hx, 0)][0:NCLS, c0:c0 + cl],
                            start=True,
                            stop=True,
                        )
                        for jt in range(5):
                            njs = NT[jt]
                            lhs = (v0_r[:, h * Dh:h * Dh + Dh + 1] if jt == 0
                                   else v_r[0:njs, jt, h * Dh:h * Dh + Dh + 1])
                            nc.tensor.matmul(
                                p2[0:Dh + 1, 0:cl],
                                lhs,
                                et_tiles[(hx, jt)][0:njs, c0:c0 + cl],
                                start=(jt == 0),
                                stop=(jt == 4),
                            )
                        bcc, bcp = bcs[2 * hx], bcs[2 * hx + 1]
                        tm = tmpp.tile([128, 298], f32, tag="tmp")
                        nc.vector.tensor_tensor(
                            tm[0:Dh, 0:cl], p2[0:Dh, 0:cl], bcp[0:Dh, 0:cl], MULT
                        )
                        if hx == 0:
                            oslice = outT[0:Dh, hq, c0:c0 + cl]
                            nc.vector.tensor_tensor(
                                oslice, p1[0:Dh, 0:cl], bcc[0:Dh, 0:cl], MULT
                            )
                            nc.vector.tensor_add(oslice, oslice, tm[0:Dh, 0:cl])
                        else:
                            tm2 = tmpp.tile([128, 298], f32, tag="tmp2")
                            nc.vector.tensor_tensor(
                                tm2[0:Dh, 0:cl], p1[0:Dh, 0:cl], bcc[0:Dh, 0:cl], MULT
                            )
                            nc.vector.tensor_add(
                                tm2[0:Dh, 0:cl], tm2[0:Dh, 0:cl], tm[0:Dh, 0:cl]
                            )
                            nc.sync.dma_start(
                                outT[64:128, hq, c0:c0 + cl], tm2[0:Dh, 0:cl]
                            )

            # ---- proj: y = out^T.T @ w_proj (natural layout out) ----
            # round outT slices to fp32r on the fly for the matmul lhsT
            for it in range(5):
                ns = NT[it]
                y = ypool.tile([128, C], f32, tag="y")
                lhs_tiles = []
                for kk in range(CT):
                    lr = lhsr.tile([128, 128], f32r, tag="lhsr")
                    nc.vector.tensor_copy(
                        lr[:, 0:ns], outT[:, kk, NSTART[it]:NSTART[it] + ns]
                    )
                    lhs_tiles.append(lr)
                for (p0, pl) in PCH:
                    pt = ps.tile([128, 384], f32, tag="ps")
                    for kk in range(CT):
                        nc.tensor.matmul(
                            pt[0:ns, 0:pl],
                            lhs_tiles[kk][:, 0:ns],
                            wp_r[:, kk, p0:p0 + pl],
                            start=(kk == 0),
                            stop=(kk == CT - 1),
                        )
                    nc.scalar.copy(y[0:ns, p0:p0 + pl], pt[0:ns, 0:pl])
                nc.sync.dma_start(out_d[b, NSTART[it]:NSTART[it] + ns, :], y[0:ns, :])


def build_program():
    nc = bacc.Bacc()
    x_d = nc.declare_dram_parameter("x", [BL, N, C], f32, isOutput=False)
    wqkv_d = nc.declare_dram_parameter("w_qkv", [C, 3 * C], f32, isOutput=False)
    bqkv_d = nc.declare_dram_parameter("b_qkv", [3 * C], f32, isOutput=False)
    wproj_d = nc.declare_dram_parameter("w_proj", [C, C], f32, isOutput=False)
    sel_d = nc.declare_dram_parameter("sel", [4, 4, 128], f32, isOutput=False)
    out_d = nc.declare_dram_parameter("out", [BL, N, C], f32, isOutput=True)
    weights_d = nc.declare_dram_parameter("weights", [BL, H, N, N], f32, isOutput=True)

    with tile.TileContext(nc) as tc:
        _emit(nc, tc, x_d, wqkv_d, bqkv_d, wproj_d, sel_d, out_d, weights_d)
    nc.compile()
    return nc


_PROGRAM = None


def _get_program():
    global _PROGRAM
    if _PROGRAM is None:
        _PROGRAM = build_program()
    return _PROGRAM


def run(x, w_qkv, b_qkv, w_proj, b_proj, trace=False, **trace_kwargs):
    """Run on 8 cores; returns (out, weights, BassKernelResults)."""
    x = np.ascontiguousarray(np.asarray(x, dtype=np.float32))
    w_qkv = np.ascontiguousarray(np.asarray(w_qkv, dtype=np.float32))
    b_qkv = np.ascontiguousarray(np.asarray(b_qkv, dtype=np.float32))
    w_proj = np.ascontiguousarray(np.asarray(w_proj, dtype=np.float32))
    b_proj = np.ascontiguousarray(np.asarray(b_proj, dtype=np.float32))

    nc = _get_program()
    sel = np.zeros((4, 4, 128), dtype=np.float32)
    for g in range(4):
        sel[g, g, :] = 1.0
    core_ids = list(range(NCORES))
    in_maps = [
        {
            "x": x[i * BL:(i + 1) * BL],
            "w_qkv": w_qkv,
            "b_qkv": b_qkv,
            "w_proj": w_proj,
            "sel": sel,
        }
        for i in core_ids
    ]
    res = run_bass_kernel_spmd(nc, in_maps, core_ids, trace=trace, **trace_kwargs)
    out = np.concatenate([r["out"] for r in res.results], axis=0)
    weights = np.concatenate([r["weights"] for r in res.results], axis=0)
    # host-side fold of the v-bias and proj bias (see module docstring)
    bias = 2.0 * (b_qkv[2 * C:] @ w_proj) + b_proj
    if np.any(bias != 0.0):
        out = out + bias[None, None, :]
    return out, weights, res


def kernel(x, w_qkv, b_qkv, w_proj, b_proj, num_heads, num_cls):
    assert int(num_heads) == H and int(num_cls) == NCLS, (num_heads, num_cls)
    out, weights, _ = run(x, w_qkv, b_qkv, w_proj, b_proj)
    return out, weights
